# revision 49
# baseline (speedup 1.0000x reference)
"""Trainium2 Bass kernel for the ChiralEmbeddingModel problem.

Pure data-parallel over 8 NeuronCores; node axis sharded, weights replicated.
Norm scales folded into weights on the host; the equivariant RMS norm cancels
through the LayerNorm (O(1e-6) EPS perturbation) and is dropped.

Device kernel is channel-major (channels on partitions, nodes on the free
axis); bf16 everywhere except PSUM accumulation.  Structure:

Pass 1 (16 pairs of 512-node tiles, 1024-wide ops):
  - 9 wide (1024-col) matmuls produce x0/y1/y2 per pair (PE).
  - PSUM->SBUF bf16 copies split across ACT and Pool engines.
  - The cross+dot runs as BATCHED DVE tensor ops: y2 is stored
    component-SHIFTED (slot j holds comp (j+2)%3) so both halves of the
    cofactor products P_i = y1_{i+1} y2_{i+2}, Q_i = y1_{i+2} y2_{i+1} are
    unit-stride 2-component chunks; M = P - Q and D = x0 * M are single
    3072-wide ops; ps = D0+D1+D2.
  - Square for the LN second moment runs lagged one pair on Pool (the only
    engine with idle time; GPSIMD cannot touch PSUM, so it gets SBUF-only
    work).  LayerNorm stats for ALL 32 tiles accumulate into one [32,512]
    PSUM pair via one-hot matmuls, emitted one pair late so PE's in-order
    queue never stalls on the DVE chain.
Rows phase (once): mean/inv-std rows on [32,512]; single Sqrt table switch
(the tanh-sigmoid gate keeps Silu/Tanh/Copy in one activation table).
Pass 2 (32 tiles, 512-wide), two stages with a 2-tile software lag:
  - gate MLP (matmuls + Silu/Tanh on ACT) runs 1-2 pairs ahead,
  - stage A: one-hot row broadcasts of -mu/K and 1/sd into PSUM (PE),
    psn = ps + mu_b (DVE), pgt = (th+1)*psn (DVE STT), gp = pgt*r (DVE,
    applied at K-width, before the out matmul),
  - stage B: out matmuls (PE), PSUM drains alternating ACT/DVE, DMA out.
All matmuls are <=512 moving columns (hardware ISA limit).
"""

import numpy as np
import ml_dtypes

import concourse.bass as bass
import concourse.tile as tile
from concourse import bacc, mybir
from concourse.bass_utils import run_bass_kernel_spmd

BF16 = ml_dtypes.bfloat16

N_ATOMS = 131072
C = 128          # equivariant channels
K = 128          # pseudoscalar dim
INV = 128        # invariant dim
OUT = 256        # output dim
HID = 2 * INV
EPS = 1e-5
N_CORES = 8
N_SHARD = N_ATOMS // N_CORES     # 16384 nodes per core
TILE = 512                       # nodes per matmul tile
N_TILES = N_SHARD // TILE        # 32
N_PAIRS = N_TILES // 2           # 16

F32 = mybir.dt.float32
BF = mybir.dt.bfloat16
AF = mybir.ActivationFunctionType
ALU = mybir.AluOpType


def _wide(ap):
    return ap.rearrange("p a b -> p (a b)")


def _build_module():
    nc = bacc.Bacc("TRN2", target_bir_lowering=False, debug=False,
                   num_devices=N_CORES)

    x_cm = nc.dram_tensor("x_cm", [C, N_TILES, 4, TILE], BF,
                          kind="ExternalInput").ap()
    a0 = nc.dram_tensor("a0", [C, C], BF, kind="ExternalInput").ap()
    a1 = nc.dram_tensor("a1", [C, C], BF, kind="ExternalInput").ap()
    a2 = nc.dram_tensor("a2", [C, C], BF, kind="ExternalInput").ap()
    g1 = nc.dram_tensor("g1", [INV, HID], BF, kind="ExternalInput").ap()
    b1 = nc.dram_tensor("b1", [INV, 2], F32, kind="ExternalInput").ap()
    g2 = nc.dram_tensor("g2", [2, INV, K], BF, kind="ExternalInput").ap()
    wo = nc.dram_tensor("wo", [K, OUT], BF, kind="ExternalInput").ap()
    ohc = nc.dram_tensor("ohc", [C, N_TILES * N_TILES], BF,
                         kind="ExternalInput").ap()
    ohr_mu = nc.dram_tensor("ohr_mu", [N_TILES, N_TILES * C], BF,
                            kind="ExternalInput").ap()
    ohr_r = nc.dram_tensor("ohr_r", [N_TILES, N_TILES * C], BF,
                           kind="ExternalInput").ap()
    # output: [2, C, N_TILES, TILE]  (o = h*128 + p)
    out_d = nc.dram_tensor("out", [2, C, N_TILES, TILE], F32,
                           kind="ExternalOutput").ap()

    with tile.TileContext(nc) as tc:
        with (
            tc.tile_pool(name="consts", bufs=1) as cp,
            tc.tile_pool(name="inp", bufs=3) as ip,
            tc.tile_pool(name="work", bufs=2) as wp,
            tc.tile_pool(name="keep", bufs=1) as kp,
        ):
            # ---- constants ----
            a0_t = cp.tile([C, C], BF, tag="a0")
            a1_t = cp.tile([C, C], BF, tag="a1")
            a2_t = cp.tile([C, C], BF, tag="a2")
            g1_t = cp.tile([INV, HID], BF, tag="g1")
            b1_t = cp.tile([INV, 2], F32, tag="b1")
            g2_t = cp.tile([INV, 2, K], BF, tag="g2")
            wo_t = cp.tile([K, OUT], BF, tag="wo")
            ohc_t = cp.tile([C, N_TILES, N_TILES], BF, tag="ohc")
            ohr_mu_t = cp.tile([N_TILES, N_TILES, C], BF, tag="ohr_mu")
            ohr_r_t = cp.tile([N_TILES, N_TILES, C], BF, tag="ohr_r")
            nc.scalar.dma_start(a0_t[:], a0[:])
            nc.scalar.dma_start(a1_t[:], a1[:])
            nc.scalar.dma_start(a2_t[:], a2[:])
            nc.scalar.dma_start(g1_t[:], g1[:])
            nc.scalar.dma_start(b1_t[:], b1[:])
            nc.scalar.dma_start(g2_t[:], g2.rearrange("h p k -> p h k"))
            nc.scalar.dma_start(wo_t[:], wo[:])
            nc.scalar.dma_start(
                ohc_t[:], ohc.rearrange("p (a b) -> p a b", a=N_TILES))
            nc.scalar.dma_start(
                ohr_mu_t[:], ohr_mu.rearrange("p (a b) -> p a b", a=N_TILES))
            nc.scalar.dma_start(
                ohr_r_t[:], ohr_r.rearrange("p (a b) -> p a b", a=N_TILES))

            eps4_t = cp.tile([N_TILES, 1], F32, tag="eps4")
            nc.vector.memset(eps4_t[:], 4.0 * EPS)

            ps_keep = []
            th_keep = []

            ps1 = tc.tile_pool(name="psum1", bufs=1, space="PSUM")
            pp = ps1.__enter__()
            # LN stats accumulators for all 32 tiles: [32, 512] PSUM
            s1_acc = pp.tile([N_TILES, TILE], F32, tag="s1acc", bufs=1)
            s2_acc = pp.tile([N_TILES, TILE], F32, tag="s2acc", bufs=1)

            # ================= PASS 1 =================
            # software pipeline: sq+stats for pair pg are emitted after pair
            # pg+1's matmuls, so PE never stalls on the DVE chain.
            def compute_pair(pg):
                t0 = 2 * pg
                # ---- load input pair: [128, 2, (inv|ex|ey|ez), TILE] ----
                in_sb = ip.tile([C, 2, 4, TILE], BF, tag="in")
                nc.sync.dma_start(in_sb[:], x_cm[:, t0:t0 + 2, :, :])

                def eq_w(i):          # wide rhs [C, 2, TILE] for component i
                    return in_sb[:, :, 1 + i, :]

                inv_w = in_sb[:, :, 0, :]

                # ---- wide matmuls x0/y1/y2 (1024 cols each) + copies ----
                # mm order + copy engine tuned so the DVE chain unblocks
                # early; y2 comp i lands in slot (i+1)%3 (see header).
                x0b = wp.tile([C, 3, 2, TILE], BF, tag="x0b")
                y1b = wp.tile([C, 3, 2, TILE], BF, tag="y1b")
                y2p = wp.tile([C, 3, 2, TILE], BF, tag="y2p")
                # GPSIMD cannot touch PSUM -> copies go to ACT (7) + DVE (2)
                plan = [
                    (a1_t, 1, y1b[:, 1, :, :], "act"),
                    (a2_t, 2, y2p[:, 0, :, :], "act"),
                    (a1_t, 2, y1b[:, 2, :, :], "act"),
                    (a2_t, 0, y2p[:, 1, :, :], "act"),
                    (a2_t, 1, y2p[:, 2, :, :], "act"),
                    (a1_t, 0, y1b[:, 0, :, :], "act"),
                    (a0_t, 0, x0b[:, 0, :, :], "act"),
                    (a0_t, 1, x0b[:, 1, :, :], "act"),
                    (a0_t, 2, x0b[:, 2, :, :], "dve"),
                ]
                dve_copies = []
                for w_t, i, dst, eng in plan:
                    ps_mm = pp.tile([C, 2, TILE], F32, tag="xyz", bufs=3)
                    for u in range(2):
                        nc.tensor.matmul(ps_mm[:, u, :], w_t[:],
                                         in_sb[:, u, 1 + i, :],
                                         start=True, stop=True)
                    if eng == "act":
                        nc.scalar.copy(_wide(dst), _wide(ps_mm))
                    else:
                        dve_copies.append((dst, ps_mm))

                # ---- batched cross + dot on DVE ----
                # P_i = y1_{i+1} * y2_{i+2};  Q_i = y1_{i+2} * y2_{i+1}
                # y2p slot j holds comp (j+2)%3, so:
                #   P[0:2] = y1[1:3] * y2p[0:2];  P[2] = y1[0] * y2p[2]
                #   Q[1:3] = y1[0:2] * y2p[0:2];  Q[0] = y1[2] * y2p[2]
                pt = wp.tile([C, 3, 2, TILE], BF, tag="pt", bufs=2)
                qt = wp.tile([C, 3, 2, TILE], BF, tag="qt", bufs=2)
                nc.vector.tensor_mul(pt[:, 0:2, :, :], y1b[:, 1:3, :, :],
                                     y2p[:, 0:2, :, :])
                nc.vector.tensor_mul(qt[:, 1:3, :, :], y1b[:, 0:2, :, :],
                                     y2p[:, 0:2, :, :])
                nc.vector.tensor_mul(qt[:, 0, :, :], y1b[:, 2, :, :],
                                     y2p[:, 2, :, :])
                nc.vector.tensor_mul(pt[:, 2, :, :], y1b[:, 0, :, :],
                                     y2p[:, 2, :, :])
                # x0 drain copies sit after P12/Q12 in the DVE queue
                for dst, ps_mm in dve_copies:
                    nc.vector.tensor_copy(_wide(dst), _wide(ps_mm))
                # in-place on DVE (in-order engine): M = P - Q into pt,
                # then D = x0 * M into qt
                mt = pt
                nc.vector.tensor_sub(mt[:], pt[:], qt[:])
                dt = qt
                nc.vector.tensor_mul(dt[:], x0b[:], mt[:])
                s01 = wp.tile([C, 2, TILE], BF, tag="s01", bufs=1)
                nc.vector.tensor_add(_wide(s01), _wide(dt[:, 0, :, :]),
                                     _wide(dt[:, 1, :, :]))
                ps_sb = kp.tile([C, 2, TILE], BF, tag=f"ps{pg}")
                nc.vector.tensor_add(_wide(ps_sb), _wide(s01),
                                     _wide(dt[:, 2, :, :]))
                ps_keep.append(ps_sb)

            def emit_stats(pg):
                # sq + one-hot stats matmuls for pair pg (lagged one pair)
                t0 = 2 * pg
                sq_sb = wp.tile([C, 2, TILE], BF, tag="sq")
                nc.gpsimd.tensor_mul(_wide(sq_sb), _wide(ps_keep[pg]),
                                     _wide(ps_keep[pg]))
                for u in range(2):
                    t_idx = t0 + u
                    nc.tensor.matmul(s1_acc[:], ohc_t[:, t_idx, :],
                                     ps_keep[pg][:, u, :],
                                     start=(t_idx == 0),
                                     stop=(t_idx == N_TILES - 1))
                    nc.tensor.matmul(s2_acc[:], ohc_t[:, t_idx, :],
                                     sq_sb[:, u, :],
                                     start=(t_idx == 0),
                                     stop=(t_idx == N_TILES - 1))

            for pg in range(N_PAIRS):
                compute_pair(pg)
                if pg >= 1:
                    emit_stats(pg - 1)
            emit_stats(N_PAIRS - 1)

            # ================= ROWS PHASE (once) =================
            s1_sb = cp.tile([N_TILES, TILE], BF, tag="s1")
            nc.vector.tensor_copy(s1_sb[:], s1_acc[:])
            m2 = wp.tile([N_TILES, TILE], F32, tag="m2")
            nc.vector.scalar_tensor_tensor(m2[:], s1_acc[:], 1.0 / (K * K),
                                           s1_sb[:], op0=ALU.mult,
                                           op1=ALU.mult)
            var = wp.tile([N_TILES, TILE], F32, tag="var")
            nc.vector.scalar_tensor_tensor(var[:], s2_acc[:], 1.0 / K,
                                           m2[:], op0=ALU.mult,
                                           op1=ALU.subtract)
            # sd' = sqrt(4*(var+eps)) = 2*sd ; rr = 1/sd' = 0.5/sd
            # (the 0.5 of the tanh-sigmoid gate rides in rr)
            sd = wp.tile([N_TILES, TILE], F32, tag="sd")
            nc.scalar.activation(sd[:], var[:], AF.Sqrt, bias=eps4_t[:],
                                 scale=4.0)
            rr = wp.tile([N_TILES, TILE], F32, tag="rr")
            nc.vector.reciprocal_approx_fast(rr[:], sd[:])
            rr_b = cp.tile([N_TILES, TILE], BF, tag="rr_b")
            nc.vector.tensor_copy(rr_b[:], rr[:])
            ps1.__exit__(None, None, None)

            ps2 = tc.tile_pool(name="psum2", bufs=1, space="PSUM")
            pp = ps2.__enter__()

            # ================= PASS 2 =================
            # gate MLP for pair pg runs one pair AHEAD of the apply chain so
            # ACT's silu/tanh never delay the out-drain copies.
            th_keep = {}

            h_q = {}

            def emit_gate_h(pg):
                t0 = 2 * pg
                inv_sb = ip.tile([C, 2, TILE], BF, tag="inv2")
                nc.sync.dma_start(inv_sb[:], x_cm[:, t0:t0 + 2, 0, :])
                h_sb = wp.tile([INV, 2, 2, TILE], BF, tag="h", bufs=3)
                for half in range(2):
                    h_ps = pp.tile([INV, 2, TILE], F32, tag="gate2h", bufs=1)
                    for u in range(2):
                        nc.tensor.matmul(h_ps[:, u, :],
                                         g1_t[:, half * INV:(half + 1) * INV],
                                         inv_sb[:, u, :],
                                         start=True, stop=True)
                    nc.scalar.activation(_wide(h_sb[:, half, :, :]),
                                         _wide(h_ps), AF.Silu,
                                         bias=b1_t[:, half:half + 1])
                h_q[pg] = h_sb

            def emit_gate_g(pg):
                h_sb = h_q.pop(pg)
                g_ps = pp.tile([K, 2, TILE], F32, tag="gate2g", bufs=1)
                for u in range(2):
                    nc.tensor.matmul(g_ps[:, u, :], g2_t[:, 0, :],
                                     h_sb[:, 0, u, :],
                                     start=True, stop=False)
                    nc.tensor.matmul(g_ps[:, u, :], g2_t[:, 1, :],
                                     h_sb[:, 1, u, :],
                                     start=False, stop=True)
                th = kp.tile([K, 2, TILE], BF, tag=f"th{pg % 3}")
                nc.scalar.activation(_wide(th), _wide(g_ps), AF.Tanh,
                                     scale=0.5)
                th_keep[pg] = th

            # two-stage apply with a 2-tile lag so PE's in-order queue never
            # serializes the chain (out mms of tile t sit behind mu/r mms of
            # t+2, whose operands are long ready).
            gp_q = {}

            def apply_a(t):
                pg, u = divmod(t, 2)
                ps_t = ps_keep[pg][:, u, :]
                th_t = th_keep[pg][:, u, :]

                mu_ps = pp.tile([C, TILE], F32, tag="mups", bufs=1)
                nc.tensor.matmul(mu_ps[:], ohr_mu_t[:, t, :], s1_sb[:],
                                 start=True, stop=True)
                r_ps = pp.tile([C, TILE], F32, tag="rps", bufs=1)
                nc.tensor.matmul(r_ps[:], ohr_r_t[:, t, :], rr_b[:],
                                 start=True, stop=True)

                # psn = ps + mu_b  (mu rows carry -1/K)  [DVE: PSUM operand]
                psn = wp.tile([C, TILE], BF, tag="psn")
                nc.vector.tensor_add(psn[:], ps_t, mu_ps[:])
                # pgt = (th+1) * psn   [DVE STT]
                pgt = wp.tile([C, TILE], BF, tag="pgt")
                nc.vector.scalar_tensor_tensor(pgt[:], th_t, 1.0, psn[:],
                                               op0=ALU.add, op1=ALU.mult)
                # gp = pgt * r  at K-width (cheaper than post-matmul) [DVE]
                gp = wp.tile([C, TILE], BF, tag="gp", bufs=3)
                nc.vector.tensor_mul(gp[:], pgt[:], r_ps[:])
                gp_q[t] = gp

            def apply_b(t):
                gp = gp_q.pop(t)
                for half in range(2):
                    o_ps = pp.tile([C, TILE], F32, tag="ops", bufs=2)
                    nc.tensor.matmul(o_ps[:],
                                     wo_t[:, half * C:(half + 1) * C],
                                     gp[:], start=True, stop=True)
                    outr = wp.tile([C, TILE], F32, tag="outr", bufs=3)
                    if half == 1:
                        nc.vector.tensor_copy(outr[:], o_ps[:])
                    else:
                        nc.scalar.copy(outr[:], o_ps[:])
                    nc.sync.dma_start(out_d[half, :, t, :], outr[:])

            emit_gate_h(0)
            emit_gate_g(0)
            emit_gate_h(1)
            emit_gate_g(1)
            apply_a(0)
            apply_a(1)
            for t in range(N_TILES):
                if t % 2 == 0:
                    if t // 2 + 2 < N_PAIRS:
                        emit_gate_h(t // 2 + 2)
                    if t // 2 + 1 < N_PAIRS and t // 2 + 1 >= 2:
                        emit_gate_g(t // 2 + 1)
                if t + 2 < N_TILES:
                    apply_a(t + 2)
                apply_b(t)
            ps2.__exit__(None, None, None)

    nc.compile()
    return nc


_NC_CACHE = None


def _get_module():
    global _NC_CACHE
    if _NC_CACHE is None:
        _NC_CACHE = _build_module()
    return _NC_CACHE


def _fold_weights(inputs):
    f32 = np.float32
    gam = inputs["rms_gamma"].astype(f32)
    s_lin = 1.0 / np.sqrt(C)
    a0 = (gam[:, None] * inputs["W_lin0"] * s_lin).astype(BF16)
    a1 = ((gam[:, None] * inputs["W_lin1"] * s_lin) @ inputs["w_cross"].T
          * (1.0 / np.sqrt(2.0 * C))).astype(BF16)
    a2 = ((gam[:, None] * inputs["W_lin2"] * s_lin) @ inputs["w_dot"].T
          * (1.0 / np.sqrt(3.0 * C))).astype(BF16)
    g1 = (inputs["gate_W1"] / inputs["std_inv"][:, None]).astype(BF16)
    b1r = ((-inputs["mean_inv"] / inputs["std_inv"]) @ inputs["gate_W1"]).astype(f32)
    b1 = np.ascontiguousarray(b1r.reshape(2, INV).T).astype(f32)   # [INV, 2]
    g2 = np.ascontiguousarray(inputs["gate_W2"].reshape(2, INV, K)).astype(BF16)
    wo = (inputs["ln_gamma"][:, None] * inputs["W_out"]).astype(BF16)
    return a0, a1, a2, g1, b1, g2, wo


def _onehot_consts():
    # ohc[:, t, m] = 1.0 iff m == t : lhsT [C, 32] slab t sums partitions
    # into stats row t (other rows accumulate +0)
    ohc = np.zeros((C, N_TILES * N_TILES), np.float32)
    for t in range(N_TILES):
        ohc[:, t * N_TILES + t] = 1.0
    # ohr[g, t*C + c] = v if g == t : lhsT broadcasting row t of rhs
    ohr_mu = np.zeros((N_TILES, N_TILES * C), np.float32)
    ohr_r = np.zeros((N_TILES, N_TILES * C), np.float32)
    for t in range(N_TILES):
        ohr_mu[t, t * C:(t + 1) * C] = -1.0 / K
        ohr_r[t, t * C:(t + 1) * C] = 1.0
    return ohc.astype(BF16), ohr_mu.astype(BF16), ohr_r.astype(BF16)


_PERM = np.concatenate([
    np.arange(INV),
    INV + 3 * np.arange(C),
    INV + 1 + 3 * np.arange(C),
    INV + 2 + 3 * np.arange(C),
])


def _make_in_maps(inputs):
    a0, a1, a2, g1, b1, g2, wo = _fold_weights(inputs)
    ohc, ohr_mu, ohr_r = _onehot_consts()

    x = np.asarray(inputs["atomic_embeddings"], dtype=np.float32)
    x_cm = x.T[_PERM]                                  # [512, N_ATOMS] view

    consts = {"a0": a0, "a1": a1, "a2": a2, "g1": g1, "b1": b1,
              "g2": g2, "wo": wo, "ohc": ohc,
              "ohr_mu": ohr_mu, "ohr_r": ohr_r}
    in_maps = []
    for c in range(N_CORES):
        m = dict(consts)
        shard = x_cm[:, c * N_SHARD:(c + 1) * N_SHARD]         # [4*C, N_SHARD]
        # -> [C, N_TILES, 4, TILE] bf16: per partition, per tile, 4KB contig
        m["x_cm"] = np.ascontiguousarray(
            shard.reshape(4, C, N_TILES, TILE).transpose(1, 2, 0, 3)).astype(BF16)
        in_maps.append(m)
    return in_maps


def kernel(**inputs):
    nc = _get_module()
    in_maps = _make_in_maps(inputs)
    res = run_bass_kernel_spmd(nc, in_maps, core_ids=list(range(N_CORES)))
    outs = []
    for r in res.results:
        o = r["out"]                                   # [2, C, N_TILES, TILE]
        outs.append(o.transpose(2, 3, 0, 1).reshape(N_SHARD, OUT))
    return np.ascontiguousarray(np.concatenate(outs, axis=0)).astype(np.float32)


# revision 57
# speedup vs baseline: 1.0205x; 1.0205x over previous
"""Trainium2 Bass kernel for the ChiralEmbeddingModel problem.

Pure data-parallel over 8 NeuronCores; node axis sharded, weights replicated.
Norm scales folded into weights on the host; the equivariant RMS norm cancels
through the LayerNorm (O(1e-6) EPS perturbation) and is dropped.

Device kernel is channel-major (channels on partitions, nodes on the free
axis); bf16 everywhere except PSUM accumulation.  Structure:

Pass 1 (16 pairs of 512-node tiles, 1024-wide ops):
  - 9 wide (1024-col) matmuls produce x0/y1/y2 per pair (PE).
  - PSUM->SBUF bf16 copies split across ACT and Pool engines.
  - The cross+dot runs as BATCHED DVE tensor ops: y2 is stored
    component-SHIFTED (slot j holds comp (j+2)%3) so both halves of the
    cofactor products P_i = y1_{i+1} y2_{i+2}, Q_i = y1_{i+2} y2_{i+1} are
    unit-stride 2-component chunks; M = P - Q and D = x0 * M are single
    3072-wide ops; ps = D0+D1+D2.
  - Square for the LN second moment runs lagged one pair on Pool (the only
    engine with idle time; GPSIMD cannot touch PSUM, so it gets SBUF-only
    work).  LayerNorm stats for ALL 32 tiles accumulate into one [32,512]
    PSUM pair via one-hot matmuls, emitted one pair late so PE's in-order
    queue never stalls on the DVE chain.
Rows phase (once): mean/inv-std rows on [32,512]; single Sqrt table switch
(the tanh-sigmoid gate keeps Silu/Tanh/Copy in one activation table).
Pass 2 (32 tiles, 512-wide), two stages with a 2-tile software lag:
  - gate MLP (matmuls + Silu/Tanh on ACT) runs 1-2 pairs ahead,
  - stage A: one-hot row broadcasts of -mu/K and 1/sd into PSUM (PE),
    psn = ps + mu_b (DVE), pgt = (th+1)*psn (DVE STT), gp = pgt*r (DVE,
    applied at K-width, before the out matmul),
  - stage B: out matmuls (PE), PSUM drains alternating ACT/DVE, DMA out.
All matmuls are <=512 moving columns (hardware ISA limit).
"""

import numpy as np
import ml_dtypes

import concourse.bass as bass
import concourse.tile as tile
from concourse import bacc, mybir
from concourse.bass_utils import run_bass_kernel_spmd

BF16 = ml_dtypes.bfloat16

N_ATOMS = 131072
C = 128          # equivariant channels
K = 128          # pseudoscalar dim
INV = 128        # invariant dim
OUT = 256        # output dim
HID = 2 * INV
EPS = 1e-5
N_CORES = 8
N_SHARD = N_ATOMS // N_CORES     # 16384 nodes per core
TILE = 512                       # nodes per matmul tile
N_TILES = N_SHARD // TILE        # 32
N_PAIRS = N_TILES // 2           # 16

F32 = mybir.dt.float32
BF = mybir.dt.bfloat16
AF = mybir.ActivationFunctionType
ALU = mybir.AluOpType


def _wide(ap):
    return ap.rearrange("p a b -> p (a b)")


def _build_module():
    nc = bacc.Bacc("TRN2", target_bir_lowering=False, debug=False,
                   num_devices=N_CORES)

    x_cm = nc.dram_tensor("x_cm", [C, N_TILES, 4, TILE], BF,
                          kind="ExternalInput").ap()
    a0 = nc.dram_tensor("a0", [C, C], BF, kind="ExternalInput").ap()
    a1 = nc.dram_tensor("a1", [C, C], BF, kind="ExternalInput").ap()
    a2 = nc.dram_tensor("a2", [C, C], BF, kind="ExternalInput").ap()
    g1 = nc.dram_tensor("g1", [INV, HID], BF, kind="ExternalInput").ap()
    b1 = nc.dram_tensor("b1", [INV, 2], F32, kind="ExternalInput").ap()
    g2 = nc.dram_tensor("g2", [2, INV, K], BF, kind="ExternalInput").ap()
    wo = nc.dram_tensor("wo", [K, OUT], BF, kind="ExternalInput").ap()
    ohc = nc.dram_tensor("ohc", [C, N_TILES * N_TILES], BF,
                         kind="ExternalInput").ap()
    ohr_mu = nc.dram_tensor("ohr_mu", [N_TILES, N_TILES * C], BF,
                            kind="ExternalInput").ap()
    ohr_r = nc.dram_tensor("ohr_r", [N_TILES, N_TILES * C], BF,
                           kind="ExternalInput").ap()
    # output: [2, C, N_TILES, TILE]  (o = h*128 + p)
    out_d = nc.dram_tensor("out", [2, C, N_TILES, TILE], F32,
                           kind="ExternalOutput").ap()
    # scratch rows for the mu/r per-node broadcasts (written after the rows
    # phase, broadcast-read per pair with a stride-0 partition DMA)
    rows_d = nc.dram_tensor("rows_scratch", [2, N_TILES, TILE], BF,
                            kind="Internal").ap()

    with tile.TileContext(nc) as tc:
        with (
            tc.tile_pool(name="consts", bufs=1) as cp,
            tc.tile_pool(name="inp", bufs=3) as ip,
            tc.tile_pool(name="work", bufs=2) as wp,
            tc.tile_pool(name="keep", bufs=1) as kp,
        ):
            # ---- constants ----
            a0_t = cp.tile([C, C], BF, tag="a0")
            a1_t = cp.tile([C, C], BF, tag="a1")
            a2_t = cp.tile([C, C], BF, tag="a2")
            g1_t = cp.tile([INV, HID], BF, tag="g1")
            b1_t = cp.tile([INV, 2], F32, tag="b1")
            g2_t = cp.tile([INV, 2, K], BF, tag="g2")
            wo_t = cp.tile([K, OUT], BF, tag="wo")
            ohc_t = cp.tile([C, N_TILES, N_TILES], BF, tag="ohc")
            nc.scalar.dma_start(a0_t[:], a0[:])
            nc.scalar.dma_start(a1_t[:], a1[:])
            nc.scalar.dma_start(a2_t[:], a2[:])
            nc.scalar.dma_start(g1_t[:], g1[:])
            nc.scalar.dma_start(b1_t[:], b1[:])
            nc.scalar.dma_start(g2_t[:], g2.rearrange("h p k -> p h k"))
            nc.scalar.dma_start(wo_t[:], wo[:])
            nc.scalar.dma_start(
                ohc_t[:], ohc.rearrange("p (a b) -> p a b", a=N_TILES))

            eps4_t = cp.tile([N_TILES, 1], F32, tag="eps4")
            nc.vector.memset(eps4_t[:], 4.0 * EPS)

            ps_keep = []
            th_keep = []

            ps1 = tc.tile_pool(name="psum1", bufs=1, space="PSUM")
            pp = ps1.__enter__()
            # LN stats accumulators for all 32 tiles: [32, 512] PSUM
            s1_acc = pp.tile([N_TILES, TILE], F32, tag="s1acc", bufs=1)
            s2_acc = pp.tile([N_TILES, TILE], F32, tag="s2acc", bufs=1)

            # ================= PASS 1 =================
            # software pipeline: sq+stats for pair pg are emitted after pair
            # pg+1's matmuls, so PE never stalls on the DVE chain.
            def compute_pair(pg):
                t0 = 2 * pg
                # ---- load input pair: [128, 2, (inv|ex|ey|ez), TILE] ----
                in_sb = ip.tile([C, 2, 4, TILE], BF, tag="in")
                nc.sync.dma_start(in_sb[:], x_cm[:, t0:t0 + 2, :, :])

                def eq_w(i):          # wide rhs [C, 2, TILE] for component i
                    return in_sb[:, :, 1 + i, :]

                inv_w = in_sb[:, :, 0, :]

                # ---- wide matmuls x0/y1/y2 (1024 cols each) + copies ----
                # mm order + copy engine tuned so the DVE chain unblocks
                # early; y2 comp i lands in slot (i+1)%3 (see header).
                x0b = wp.tile([C, 3, 2, TILE], BF, tag="x0b")
                y1b = wp.tile([C, 3, 2, TILE], BF, tag="y1b")
                y2p = wp.tile([C, 3, 2, TILE], BF, tag="y2p")
                # GPSIMD cannot touch PSUM -> copies go to ACT (7) + DVE (2)
                plan = [
                    (a1_t, 1, y1b[:, 1, :, :], "act"),
                    (a2_t, 2, y2p[:, 0, :, :], "act"),
                    (a1_t, 2, y1b[:, 2, :, :], "act"),
                    (a2_t, 0, y2p[:, 1, :, :], "act"),
                    (a2_t, 1, y2p[:, 2, :, :], "act"),
                    (a1_t, 0, y1b[:, 0, :, :], "act"),
                    (a0_t, 0, x0b[:, 0, :, :], "act"),
                    (a0_t, 1, x0b[:, 1, :, :], "act"),
                    (a0_t, 2, x0b[:, 2, :, :], "dve"),
                ]
                dve_copies = []
                for w_t, i, dst, eng in plan:
                    ps_mm = pp.tile([C, 2, TILE], F32, tag="xyz", bufs=3)
                    for u in range(2):
                        nc.tensor.matmul(ps_mm[:, u, :], w_t[:],
                                         in_sb[:, u, 1 + i, :],
                                         start=True, stop=True)
                    if eng == "act":
                        nc.scalar.copy(_wide(dst), _wide(ps_mm))
                    else:
                        dve_copies.append((dst, ps_mm))

                # ---- batched cross + dot on DVE ----
                # P_i = y1_{i+1} * y2_{i+2};  Q_i = y1_{i+2} * y2_{i+1}
                # y2p slot j holds comp (j+2)%3, so:
                #   P[0:2] = y1[1:3] * y2p[0:2];  P[2] = y1[0] * y2p[2]
                #   Q[1:3] = y1[0:2] * y2p[0:2];  Q[0] = y1[2] * y2p[2]
                pt = wp.tile([C, 3, 2, TILE], BF, tag="pt", bufs=2)
                qt = wp.tile([C, 3, 2, TILE], BF, tag="qt", bufs=2)
                nc.vector.tensor_mul(pt[:, 0:2, :, :], y1b[:, 1:3, :, :],
                                     y2p[:, 0:2, :, :])
                nc.vector.tensor_mul(qt[:, 1:3, :, :], y1b[:, 0:2, :, :],
                                     y2p[:, 0:2, :, :])
                nc.vector.tensor_mul(qt[:, 0, :, :], y1b[:, 2, :, :],
                                     y2p[:, 2, :, :])
                nc.vector.tensor_mul(pt[:, 2, :, :], y1b[:, 0, :, :],
                                     y2p[:, 2, :, :])
                # x0 drain copies sit after P12/Q12 in the DVE queue
                for dst, ps_mm in dve_copies:
                    nc.vector.tensor_copy(_wide(dst), _wide(ps_mm))
                # in-place on DVE (in-order engine): M = P - Q into pt,
                # then D = x0 * M into qt
                mt = pt
                nc.vector.tensor_sub(mt[:], pt[:], qt[:])
                dt = qt
                nc.vector.tensor_mul(dt[:], x0b[:], mt[:])
                s01 = wp.tile([C, 2, TILE], BF, tag="s01", bufs=1)
                nc.vector.tensor_add(_wide(s01), _wide(dt[:, 0, :, :]),
                                     _wide(dt[:, 1, :, :]))
                ps_sb = kp.tile([C, 2, TILE], BF, tag=f"ps{pg}")
                nc.vector.tensor_add(_wide(ps_sb), _wide(s01),
                                     _wide(dt[:, 2, :, :]))
                ps_keep.append(ps_sb)

            def emit_stats(pg):
                # sq + one-hot stats matmuls for pair pg (lagged one pair)
                t0 = 2 * pg
                sq_sb = wp.tile([C, 2, TILE], BF, tag="sq")
                nc.gpsimd.tensor_mul(_wide(sq_sb), _wide(ps_keep[pg]),
                                     _wide(ps_keep[pg]))
                for u in range(2):
                    t_idx = t0 + u
                    nc.tensor.matmul(s1_acc[:], ohc_t[:, t_idx, :],
                                     ps_keep[pg][:, u, :],
                                     start=(t_idx == 0),
                                     stop=(t_idx == N_TILES - 1))
                    nc.tensor.matmul(s2_acc[:], ohc_t[:, t_idx, :],
                                     sq_sb[:, u, :],
                                     start=(t_idx == 0),
                                     stop=(t_idx == N_TILES - 1))

            for pg in range(N_PAIRS):
                compute_pair(pg)
                if pg >= 1:
                    emit_stats(pg - 1)
            emit_stats(N_PAIRS - 1)

            # ================= ROWS PHASE (once) =================
            s1_sb = cp.tile([N_TILES, TILE], BF, tag="s1")
            nc.vector.tensor_copy(s1_sb[:], s1_acc[:])
            m2 = wp.tile([N_TILES, TILE], F32, tag="m2")
            nc.vector.scalar_tensor_tensor(m2[:], s1_acc[:], 1.0 / (K * K),
                                           s1_sb[:], op0=ALU.mult,
                                           op1=ALU.mult)
            var = wp.tile([N_TILES, TILE], F32, tag="var")
            nc.vector.scalar_tensor_tensor(var[:], s2_acc[:], 1.0 / K,
                                           m2[:], op0=ALU.mult,
                                           op1=ALU.subtract)
            # sd' = sqrt(4*(var+eps)) = 2*sd ; rr = 1/sd' = 0.5/sd
            # (the 0.5 of the tanh-sigmoid gate rides in rr)
            sd = wp.tile([N_TILES, TILE], F32, tag="sd")
            nc.scalar.activation(sd[:], var[:], AF.Sqrt, bias=eps4_t[:],
                                 scale=4.0)
            rr = wp.tile([N_TILES, TILE], F32, tag="rr")
            nc.vector.reciprocal_approx_fast(rr[:], sd[:])
            rr_b = cp.tile([N_TILES, TILE], BF, tag="rr_b")
            nc.vector.tensor_copy(rr_b[:], rr[:])
            # mu rows = -s1/K as bf16, then park both row sets in DRAM; the
            # per-pair broadcast DMAs ride the same queues, so in-queue FIFO
            # order guarantees they see the writes
            mu_rows = cp.tile([N_TILES, TILE], BF, tag="mu_rows")
            nc.scalar.activation(mu_rows[:], s1_acc[:], AF.Copy,
                                 scale=-1.0 / K)
            nc.sync.dma_start(rows_d[0, :, :], mu_rows[:])
            nc.scalar.dma_start(rows_d[1, :, :], rr_b[:])
            ps1.__exit__(None, None, None)

            ps2 = tc.tile_pool(name="psum2", bufs=1, space="PSUM")
            pp = ps2.__enter__()

            # ================= PASS 2 =================
            # gate MLP for pair pg runs one pair AHEAD of the apply chain so
            # ACT's silu/tanh never delay the out-drain copies.
            th_keep = {}

            h_q = {}

            def emit_gate_h(pg):
                t0 = 2 * pg
                inv_sb = ip.tile([C, 2, TILE], BF, tag="inv2")
                nc.gpsimd.dma_start(inv_sb[:], x_cm[:, t0:t0 + 2, 0, :])
                h_sb = wp.tile([INV, 2, 2, TILE], BF, tag="h", bufs=3)
                for half in range(2):
                    h_ps = pp.tile([INV, 2, TILE], F32, tag="gate2h", bufs=1)
                    for u in range(2):
                        nc.tensor.matmul(h_ps[:, u, :],
                                         g1_t[:, half * INV:(half + 1) * INV],
                                         inv_sb[:, u, :],
                                         start=True, stop=True)
                    nc.scalar.activation(_wide(h_sb[:, half, :, :]),
                                         _wide(h_ps), AF.Silu,
                                         bias=b1_t[:, half:half + 1])
                h_q[pg] = h_sb

            def emit_gate_g(pg):
                h_sb = h_q.pop(pg)
                g_ps = pp.tile([K, 2, TILE], F32, tag="gate2g", bufs=1)
                for u in range(2):
                    nc.tensor.matmul(g_ps[:, u, :], g2_t[:, 0, :],
                                     h_sb[:, 0, u, :],
                                     start=True, stop=False)
                    nc.tensor.matmul(g_ps[:, u, :], g2_t[:, 1, :],
                                     h_sb[:, 1, u, :],
                                     start=False, stop=True)
                th = kp.tile([K, 2, TILE], BF, tag=f"th{pg % 3}")
                nc.scalar.activation(_wide(th), _wide(g_ps), AF.Tanh,
                                     scale=0.5)
                th_keep[pg] = th

            # two-stage apply (pair-granular) with a 1-pair lag; mu/r rows
            # arrive via broadcast DMAs (stride-0 DRAM reads) as bf16 SBUF,
            # so the whole normalize chain runs at the DVE 2x rate and the
            # one-hot broadcast matmuls disappear.
            gp_q = {}

            def apply_a(pg):
                t0 = 2 * pg
                mur = ip.tile([C, 2, 2, TILE], BF, tag="mur", bufs=2)
                src = rows_d[:, t0:t0 + 2, :]
                srcb = src.unsqueeze(0).broadcast_to([C, 2, 2, TILE])
                nc.sync.dma_start(mur[:, 0, :, :], srcb[:, 0, :, :])
                nc.scalar.dma_start(mur[:, 1, :, :], srcb[:, 1, :, :])

                # psn = ps + mu_b ; pgt = (th+1)*psn ; gp = pgt*r  [all bf16]
                psn = wp.tile([C, 2, TILE], BF, tag="psn")
                nc.vector.tensor_add(_wide(psn), _wide(ps_keep[pg]),
                                     _wide(mur[:, 0, :, :]))
                pgt = wp.tile([C, 2, TILE], BF, tag="pgt")
                nc.vector.scalar_tensor_tensor(_wide(pgt),
                                               _wide(th_keep[pg]), 1.0,
                                               _wide(psn), op0=ALU.add,
                                               op1=ALU.mult)
                gp = wp.tile([C, 2, TILE], BF, tag="gp", bufs=2)
                nc.vector.tensor_mul(_wide(gp), _wide(pgt),
                                     _wide(mur[:, 1, :, :]))
                gp_q[pg] = gp

            def apply_b(pg):
                t0 = 2 * pg
                gp = gp_q.pop(pg)
                for half in range(2):
                    o_ps = pp.tile([C, 2, TILE], F32, tag="ops", bufs=2)
                    for u in range(2):
                        nc.tensor.matmul(o_ps[:, u, :],
                                         wo_t[:, half * C:(half + 1) * C],
                                         gp[:, u, :], start=True, stop=True)
                    outr = wp.tile([C, 2, TILE], F32, tag="outr", bufs=2)
                    if half == 1:
                        nc.vector.tensor_copy(_wide(outr), _wide(o_ps))
                        nc.sync.dma_start(out_d[half, :, t0:t0 + 2, :],
                                          outr[:])
                    else:
                        nc.scalar.copy(_wide(outr), _wide(o_ps))
                        nc.sync.dma_start(out_d[half, :, t0:t0 + 2, :],
                                          outr[:])

            emit_gate_h(0)
            emit_gate_g(0)
            emit_gate_h(1)
            emit_gate_g(1)
            apply_a(0)
            for pg in range(N_PAIRS):
                if pg + 2 < N_PAIRS:
                    emit_gate_h(pg + 2)
                if pg + 1 < N_PAIRS and pg + 1 >= 2:
                    emit_gate_g(pg + 1)
                if pg + 1 < N_PAIRS:
                    apply_a(pg + 1)
                apply_b(pg)
            ps2.__exit__(None, None, None)

    nc.compile()
    return nc


_NC_CACHE = None


def _get_module():
    global _NC_CACHE
    if _NC_CACHE is None:
        _NC_CACHE = _build_module()
    return _NC_CACHE


def _fold_weights(inputs):
    f32 = np.float32
    gam = inputs["rms_gamma"].astype(f32)
    s_lin = 1.0 / np.sqrt(C)
    a0 = (gam[:, None] * inputs["W_lin0"] * s_lin).astype(BF16)
    a1 = ((gam[:, None] * inputs["W_lin1"] * s_lin) @ inputs["w_cross"].T
          * (1.0 / np.sqrt(2.0 * C))).astype(BF16)
    a2 = ((gam[:, None] * inputs["W_lin2"] * s_lin) @ inputs["w_dot"].T
          * (1.0 / np.sqrt(3.0 * C))).astype(BF16)
    g1 = (inputs["gate_W1"] / inputs["std_inv"][:, None]).astype(BF16)
    b1r = ((-inputs["mean_inv"] / inputs["std_inv"]) @ inputs["gate_W1"]).astype(f32)
    b1 = np.ascontiguousarray(b1r.reshape(2, INV).T).astype(f32)   # [INV, 2]
    g2 = np.ascontiguousarray(inputs["gate_W2"].reshape(2, INV, K)).astype(BF16)
    wo = (inputs["ln_gamma"][:, None] * inputs["W_out"]).astype(BF16)
    return a0, a1, a2, g1, b1, g2, wo


def _onehot_consts():
    # ohc[:, t, m] = 1.0 iff m == t : lhsT [C, 32] slab t sums partitions
    # into stats row t (other rows accumulate +0)
    ohc = np.zeros((C, N_TILES * N_TILES), np.float32)
    for t in range(N_TILES):
        ohc[:, t * N_TILES + t] = 1.0
    # ohr[g, t*C + c] = v if g == t : lhsT broadcasting row t of rhs
    ohr_mu = np.zeros((N_TILES, N_TILES * C), np.float32)
    ohr_r = np.zeros((N_TILES, N_TILES * C), np.float32)
    for t in range(N_TILES):
        ohr_mu[t, t * C:(t + 1) * C] = -1.0 / K
        ohr_r[t, t * C:(t + 1) * C] = 1.0
    return ohc.astype(BF16), ohr_mu.astype(BF16), ohr_r.astype(BF16)


_PERM = np.concatenate([
    np.arange(INV),
    INV + 3 * np.arange(C),
    INV + 1 + 3 * np.arange(C),
    INV + 2 + 3 * np.arange(C),
])


def _make_in_maps(inputs):
    a0, a1, a2, g1, b1, g2, wo = _fold_weights(inputs)
    ohc, ohr_mu, ohr_r = _onehot_consts()

    x = np.asarray(inputs["atomic_embeddings"], dtype=np.float32)
    x_cm = x.T[_PERM]                                  # [512, N_ATOMS] view

    consts = {"a0": a0, "a1": a1, "a2": a2, "g1": g1, "b1": b1,
              "g2": g2, "wo": wo, "ohc": ohc,
              "ohr_mu": ohr_mu, "ohr_r": ohr_r}
    in_maps = []
    for c in range(N_CORES):
        m = dict(consts)
        shard = x_cm[:, c * N_SHARD:(c + 1) * N_SHARD]         # [4*C, N_SHARD]
        # -> [C, N_TILES, 4, TILE] bf16: per partition, per tile, 4KB contig
        m["x_cm"] = np.ascontiguousarray(
            shard.reshape(4, C, N_TILES, TILE).transpose(1, 2, 0, 3)).astype(BF16)
        in_maps.append(m)
    return in_maps


def kernel(**inputs):
    nc = _get_module()
    in_maps = _make_in_maps(inputs)
    res = run_bass_kernel_spmd(nc, in_maps, core_ids=list(range(N_CORES)))
    outs = []
    for r in res.results:
        o = r["out"]                                   # [2, C, N_TILES, TILE]
        outs.append(o.transpose(2, 3, 0, 1).reshape(N_SHARD, OUT))
    return np.ascontiguousarray(np.concatenate(outs, axis=0)).astype(np.float32)


# revision 59
# speedup vs baseline: 1.0219x; 1.0014x over previous
"""Trainium2 Bass kernel for the ChiralEmbeddingModel problem.

Pure data-parallel over 8 NeuronCores; node axis sharded, weights replicated.
Norm scales folded into weights on the host; the equivariant RMS norm cancels
through the LayerNorm (O(1e-6) EPS perturbation) and is dropped.

Device kernel is channel-major (channels on partitions, nodes on the free
axis); bf16 everywhere except PSUM accumulation.  Structure:

Pass 1 (16 pairs of 512-node tiles, 1024-wide ops):
  - 9 wide (1024-col) matmuls produce x0/y1/y2 per pair (PE).
  - PSUM->SBUF bf16 copies split across ACT and Pool engines.
  - The cross+dot runs as BATCHED DVE tensor ops: y2 is stored
    component-SHIFTED (slot j holds comp (j+2)%3) so both halves of the
    cofactor products P_i = y1_{i+1} y2_{i+2}, Q_i = y1_{i+2} y2_{i+1} are
    unit-stride 2-component chunks; M = P - Q and D = x0 * M are single
    3072-wide ops; ps = D0+D1+D2.
  - Square for the LN second moment runs lagged one pair on Pool (the only
    engine with idle time; GPSIMD cannot touch PSUM, so it gets SBUF-only
    work).  LayerNorm stats for ALL 32 tiles accumulate into one [32,512]
    PSUM pair via one-hot matmuls, emitted one pair late so PE's in-order
    queue never stalls on the DVE chain.
Rows phase (once): mean/inv-std rows on [32,512]; single Sqrt table switch
(the tanh-sigmoid gate keeps Silu/Tanh/Copy in one activation table).
Pass 2 (16 pairs, 1024-wide), two stages with a 1-pair software lag:
  - the -mu/K and 1/sd rows are parked in a DRAM scratch after the rows
    phase and broadcast to all 128 partitions per pair via stride-0 DMA
    reads (mu on the sync queue, r on the scalar queue -- same-queue FIFO
    order guarantees they see the row writes).  This kills the one-hot
    broadcast matmuls and lets the whole normalize chain run as bf16 SBUF
    ops at the DVE 2x rate,
  - gate MLP (matmuls + Silu/Tanh on ACT) runs 1-2 pairs ahead,
  - stage A: psn = ps + mu_b, pgt = (th+1)*psn (STT), gp = pgt*r (all DVE,
    r applied at K-width before the out matmul),
  - stage B: out matmuls (PE), PSUM drains split ACT/DVE, out-DMAs on the
    sync queue, gate-input reloads on the gpsimd (SWDGE) queue.
All matmuls are <=512 moving columns (hardware ISA limit).
"""

import numpy as np
import ml_dtypes

import concourse.bass as bass
import concourse.tile as tile
from concourse import bacc, mybir
from concourse.bass_utils import run_bass_kernel_spmd

BF16 = ml_dtypes.bfloat16

N_ATOMS = 131072
C = 128          # equivariant channels
K = 128          # pseudoscalar dim
INV = 128        # invariant dim
OUT = 256        # output dim
HID = 2 * INV
EPS = 1e-5
N_CORES = 8
N_SHARD = N_ATOMS // N_CORES     # 16384 nodes per core
TILE = 512                       # nodes per matmul tile
N_TILES = N_SHARD // TILE        # 32
N_PAIRS = N_TILES // 2           # 16

F32 = mybir.dt.float32
BF = mybir.dt.bfloat16
AF = mybir.ActivationFunctionType
ALU = mybir.AluOpType


def _wide(ap):
    return ap.rearrange("p a b -> p (a b)")


def _build_module():
    nc = bacc.Bacc("TRN2", target_bir_lowering=False, debug=False,
                   num_devices=N_CORES)

    x_cm = nc.dram_tensor("x_cm", [C, N_TILES, 4, TILE], BF,
                          kind="ExternalInput").ap()
    a0 = nc.dram_tensor("a0", [C, C], BF, kind="ExternalInput").ap()
    a1 = nc.dram_tensor("a1", [C, C], BF, kind="ExternalInput").ap()
    a2 = nc.dram_tensor("a2", [C, C], BF, kind="ExternalInput").ap()
    g1 = nc.dram_tensor("g1", [INV, HID], BF, kind="ExternalInput").ap()
    b1 = nc.dram_tensor("b1", [INV, 2], F32, kind="ExternalInput").ap()
    g2 = nc.dram_tensor("g2", [2, INV, K], BF, kind="ExternalInput").ap()
    wo = nc.dram_tensor("wo", [K, OUT], BF, kind="ExternalInput").ap()
    ohc = nc.dram_tensor("ohc", [C, N_TILES * N_TILES], BF,
                         kind="ExternalInput").ap()
    ohr_mu = nc.dram_tensor("ohr_mu", [N_TILES, N_TILES * C], BF,
                            kind="ExternalInput").ap()
    ohr_r = nc.dram_tensor("ohr_r", [N_TILES, N_TILES * C], BF,
                           kind="ExternalInput").ap()
    # output: [2, C, N_TILES, TILE]  (o = h*128 + p)
    out_d = nc.dram_tensor("out", [2, C, N_TILES, TILE], F32,
                           kind="ExternalOutput").ap()
    # scratch rows for the mu/r per-node broadcasts (written after the rows
    # phase, broadcast-read per pair with a stride-0 partition DMA)
    rows_d = nc.dram_tensor("rows_scratch", [2, N_TILES, TILE], BF,
                            kind="Internal").ap()

    with tile.TileContext(nc) as tc:
        with (
            tc.tile_pool(name="consts", bufs=1) as cp,
            tc.tile_pool(name="inp", bufs=3) as ip,
            tc.tile_pool(name="work", bufs=2) as wp,
            tc.tile_pool(name="keep", bufs=1) as kp,
        ):
            # ---- constants ----
            a0_t = cp.tile([C, C], BF, tag="a0")
            a1_t = cp.tile([C, C], BF, tag="a1")
            a2_t = cp.tile([C, C], BF, tag="a2")
            g1_t = cp.tile([INV, HID], BF, tag="g1")
            b1_t = cp.tile([INV, 2], F32, tag="b1")
            g2_t = cp.tile([INV, 2, K], BF, tag="g2")
            wo_t = cp.tile([K, OUT], BF, tag="wo")
            ohc_t = cp.tile([C, N_TILES, N_TILES], BF, tag="ohc")
            nc.scalar.dma_start(a0_t[:], a0[:])
            nc.scalar.dma_start(a1_t[:], a1[:])
            nc.scalar.dma_start(a2_t[:], a2[:])
            nc.scalar.dma_start(g1_t[:], g1[:])
            nc.scalar.dma_start(b1_t[:], b1[:])
            nc.scalar.dma_start(g2_t[:], g2.rearrange("h p k -> p h k"))
            nc.scalar.dma_start(wo_t[:], wo[:])
            nc.scalar.dma_start(
                ohc_t[:], ohc.rearrange("p (a b) -> p a b", a=N_TILES))

            eps4_t = cp.tile([N_TILES, 1], F32, tag="eps4")
            nc.vector.memset(eps4_t[:], 4.0 * EPS)

            ps_keep = []
            th_keep = []

            ps1 = tc.tile_pool(name="psum1", bufs=1, space="PSUM")
            pp = ps1.__enter__()
            # LN stats accumulators for all 32 tiles: [32, 512] PSUM
            s1_acc = pp.tile([N_TILES, TILE], F32, tag="s1acc", bufs=1)
            s2_acc = pp.tile([N_TILES, TILE], F32, tag="s2acc", bufs=1)

            # ================= PASS 1 =================
            # software pipeline: sq+stats for pair pg are emitted after pair
            # pg+1's matmuls, so PE never stalls on the DVE chain.
            def compute_pair(pg):
                t0 = 2 * pg
                # ---- load input pair: [128, 2, (inv|ex|ey|ez), TILE] ----
                in_sb = ip.tile([C, 2, 4, TILE], BF, tag="in")
                nc.sync.dma_start(in_sb[:], x_cm[:, t0:t0 + 2, :, :])

                def eq_w(i):          # wide rhs [C, 2, TILE] for component i
                    return in_sb[:, :, 1 + i, :]

                inv_w = in_sb[:, :, 0, :]

                # ---- wide matmuls x0/y1/y2 (1024 cols each) + copies ----
                # mm order + copy engine tuned so the DVE chain unblocks
                # early; y2 comp i lands in slot (i+1)%3 (see header).
                x0b = wp.tile([C, 3, 2, TILE], BF, tag="x0b")
                y1b = wp.tile([C, 3, 2, TILE], BF, tag="y1b")
                y2p = wp.tile([C, 3, 2, TILE], BF, tag="y2p")
                # GPSIMD cannot touch PSUM -> copies go to ACT (7) + DVE (2)
                plan = [
                    (a1_t, 1, y1b[:, 1, :, :], "act"),
                    (a2_t, 2, y2p[:, 0, :, :], "act"),
                    (a1_t, 2, y1b[:, 2, :, :], "act"),
                    (a2_t, 0, y2p[:, 1, :, :], "act"),
                    (a2_t, 1, y2p[:, 2, :, :], "act"),
                    (a1_t, 0, y1b[:, 0, :, :], "act"),
                    (a0_t, 0, x0b[:, 0, :, :], "act"),
                    (a0_t, 1, x0b[:, 1, :, :], "act"),
                    (a0_t, 2, x0b[:, 2, :, :], "dve"),
                ]
                dve_copies = []
                for w_t, i, dst, eng in plan:
                    ps_mm = pp.tile([C, 2, TILE], F32, tag="xyz", bufs=3)
                    for u in range(2):
                        nc.tensor.matmul(ps_mm[:, u, :], w_t[:],
                                         in_sb[:, u, 1 + i, :],
                                         start=True, stop=True)
                    if eng == "act":
                        nc.scalar.copy(_wide(dst), _wide(ps_mm))
                    else:
                        dve_copies.append((dst, ps_mm))

                # ---- batched cross + dot on DVE ----
                # P_i = y1_{i+1} * y2_{i+2};  Q_i = y1_{i+2} * y2_{i+1}
                # y2p slot j holds comp (j+2)%3, so:
                #   P[0:2] = y1[1:3] * y2p[0:2];  P[2] = y1[0] * y2p[2]
                #   Q[1:3] = y1[0:2] * y2p[0:2];  Q[0] = y1[2] * y2p[2]
                pt = wp.tile([C, 3, 2, TILE], BF, tag="pt", bufs=2)
                qt = wp.tile([C, 3, 2, TILE], BF, tag="qt", bufs=2)
                nc.vector.tensor_mul(pt[:, 0:2, :, :], y1b[:, 1:3, :, :],
                                     y2p[:, 0:2, :, :])
                nc.vector.tensor_mul(qt[:, 1:3, :, :], y1b[:, 0:2, :, :],
                                     y2p[:, 0:2, :, :])
                nc.vector.tensor_mul(qt[:, 0, :, :], y1b[:, 2, :, :],
                                     y2p[:, 2, :, :])
                nc.vector.tensor_mul(pt[:, 2, :, :], y1b[:, 0, :, :],
                                     y2p[:, 2, :, :])
                # x0 drain copies sit after P12/Q12 in the DVE queue
                for dst, ps_mm in dve_copies:
                    nc.vector.tensor_copy(_wide(dst), _wide(ps_mm))
                # in-place on DVE (in-order engine): M = P - Q into pt,
                # then D = x0 * M into qt
                mt = pt
                nc.vector.tensor_sub(mt[:], pt[:], qt[:])
                dt = qt
                nc.vector.tensor_mul(dt[:], x0b[:], mt[:])
                s01 = wp.tile([C, 2, TILE], BF, tag="s01", bufs=1)
                nc.vector.tensor_add(_wide(s01), _wide(dt[:, 0, :, :]),
                                     _wide(dt[:, 1, :, :]))
                ps_sb = kp.tile([C, 2, TILE], BF, tag=f"ps{pg}")
                nc.vector.tensor_add(_wide(ps_sb), _wide(s01),
                                     _wide(dt[:, 2, :, :]))
                ps_keep.append(ps_sb)

            def emit_stats(pg):
                # sq + one-hot stats matmuls for pair pg (lagged one pair)
                t0 = 2 * pg
                sq_sb = wp.tile([C, 2, TILE], BF, tag="sq")
                nc.gpsimd.tensor_mul(_wide(sq_sb), _wide(ps_keep[pg]),
                                     _wide(ps_keep[pg]))
                for u in range(2):
                    t_idx = t0 + u
                    nc.tensor.matmul(s1_acc[:], ohc_t[:, t_idx, :],
                                     ps_keep[pg][:, u, :],
                                     start=(t_idx == 0),
                                     stop=(t_idx == N_TILES - 1))
                    nc.tensor.matmul(s2_acc[:], ohc_t[:, t_idx, :],
                                     sq_sb[:, u, :],
                                     start=(t_idx == 0),
                                     stop=(t_idx == N_TILES - 1))

            for pg in range(N_PAIRS):
                compute_pair(pg)
                if pg >= 1:
                    emit_stats(pg - 1)
            emit_stats(N_PAIRS - 1)

            # ================= ROWS PHASE (once) =================
            s1_sb = cp.tile([N_TILES, TILE], BF, tag="s1")
            nc.vector.tensor_copy(s1_sb[:], s1_acc[:])
            m2 = wp.tile([N_TILES, TILE], F32, tag="m2")
            nc.vector.scalar_tensor_tensor(m2[:], s1_acc[:], 1.0 / (K * K),
                                           s1_sb[:], op0=ALU.mult,
                                           op1=ALU.mult)
            var = wp.tile([N_TILES, TILE], F32, tag="var")
            nc.vector.scalar_tensor_tensor(var[:], s2_acc[:], 1.0 / K,
                                           m2[:], op0=ALU.mult,
                                           op1=ALU.subtract)
            # sd' = sqrt(4*(var+eps)) = 2*sd ; rr = 1/sd' = 0.5/sd
            # (the 0.5 of the tanh-sigmoid gate rides in rr)
            sd = wp.tile([N_TILES, TILE], F32, tag="sd")
            nc.scalar.activation(sd[:], var[:], AF.Sqrt, bias=eps4_t[:],
                                 scale=4.0)
            rr = wp.tile([N_TILES, TILE], F32, tag="rr")
            nc.vector.reciprocal_approx_fast(rr[:], sd[:])
            rr_b = cp.tile([N_TILES, TILE], BF, tag="rr_b")
            nc.vector.tensor_copy(rr_b[:], rr[:])
            # mu rows = -s1/K as bf16, then park both row sets in DRAM; the
            # per-pair broadcast DMAs ride the same queues, so in-queue FIFO
            # order guarantees they see the writes
            mu_rows = cp.tile([N_TILES, TILE], BF, tag="mu_rows")
            nc.scalar.activation(mu_rows[:], s1_acc[:], AF.Copy,
                                 scale=-1.0 / K)
            nc.sync.dma_start(rows_d[0, :, :], mu_rows[:])
            nc.scalar.dma_start(rows_d[1, :, :], rr_b[:])
            ps1.__exit__(None, None, None)

            ps2 = tc.tile_pool(name="psum2", bufs=1, space="PSUM")
            pp = ps2.__enter__()

            # ================= PASS 2 =================
            # gate MLP for pair pg runs one pair AHEAD of the apply chain so
            # ACT's silu/tanh never delay the out-drain copies.
            th_keep = {}

            h_q = {}

            def emit_gate_h(pg):
                t0 = 2 * pg
                inv_sb = ip.tile([C, 2, TILE], BF, tag="inv2")
                nc.gpsimd.dma_start(inv_sb[:], x_cm[:, t0:t0 + 2, 0, :])
                h_sb = wp.tile([INV, 2, 2, TILE], BF, tag="h", bufs=3)
                for half in range(2):
                    h_ps = pp.tile([INV, 2, TILE], F32, tag="gate2h", bufs=1)
                    for u in range(2):
                        nc.tensor.matmul(h_ps[:, u, :],
                                         g1_t[:, half * INV:(half + 1) * INV],
                                         inv_sb[:, u, :],
                                         start=True, stop=True)
                    nc.scalar.activation(_wide(h_sb[:, half, :, :]),
                                         _wide(h_ps), AF.Silu,
                                         bias=b1_t[:, half:half + 1])
                h_q[pg] = h_sb

            def emit_gate_g(pg):
                h_sb = h_q.pop(pg)
                g_ps = pp.tile([K, 2, TILE], F32, tag="gate2g", bufs=1)
                for u in range(2):
                    nc.tensor.matmul(g_ps[:, u, :], g2_t[:, 0, :],
                                     h_sb[:, 0, u, :],
                                     start=True, stop=False)
                    nc.tensor.matmul(g_ps[:, u, :], g2_t[:, 1, :],
                                     h_sb[:, 1, u, :],
                                     start=False, stop=True)
                th = kp.tile([K, 2, TILE], BF, tag=f"th{pg % 3}")
                nc.scalar.activation(_wide(th), _wide(g_ps), AF.Tanh,
                                     scale=0.5)
                th_keep[pg] = th

            # two-stage apply (pair-granular) with a 1-pair lag; mu/r rows
            # arrive via broadcast DMAs (stride-0 DRAM reads) as bf16 SBUF,
            # so the whole normalize chain runs at the DVE 2x rate and the
            # one-hot broadcast matmuls disappear.
            gp_q = {}

            def apply_a(pg):
                t0 = 2 * pg
                mur = ip.tile([C, 2, 2, TILE], BF, tag="mur", bufs=3)
                src = rows_d[:, t0:t0 + 2, :]
                srcb = src.unsqueeze(0).broadcast_to([C, 2, 2, TILE])
                nc.sync.dma_start(mur[:, 0, :, :], srcb[:, 0, :, :])
                nc.scalar.dma_start(mur[:, 1, :, :], srcb[:, 1, :, :])

                # psn = ps + mu_b ; pgt = (th+1)*psn ; gp = pgt*r  [all bf16]
                psn = wp.tile([C, 2, TILE], BF, tag="psn")
                nc.vector.tensor_add(_wide(psn), _wide(ps_keep[pg]),
                                     _wide(mur[:, 0, :, :]))
                pgt = wp.tile([C, 2, TILE], BF, tag="pgt")
                nc.vector.scalar_tensor_tensor(_wide(pgt),
                                               _wide(th_keep[pg]), 1.0,
                                               _wide(psn), op0=ALU.add,
                                               op1=ALU.mult)
                gp = wp.tile([C, 2, TILE], BF, tag="gp", bufs=2)
                nc.vector.tensor_mul(_wide(gp), _wide(pgt),
                                     _wide(mur[:, 1, :, :]))
                gp_q[pg] = gp

            def apply_b(pg):
                t0 = 2 * pg
                gp = gp_q.pop(pg)
                for half in range(2):
                    o_ps = pp.tile([C, 2, TILE], F32, tag="ops", bufs=2)
                    for u in range(2):
                        nc.tensor.matmul(o_ps[:, u, :],
                                         wo_t[:, half * C:(half + 1) * C],
                                         gp[:, u, :], start=True, stop=True)
                    outr = wp.tile([C, 2, TILE], F32, tag="outr", bufs=2)
                    if half == 1:
                        nc.vector.tensor_copy(_wide(outr), _wide(o_ps))
                        nc.sync.dma_start(out_d[half, :, t0:t0 + 2, :],
                                          outr[:])
                    else:
                        nc.scalar.copy(_wide(outr), _wide(o_ps))
                        nc.sync.dma_start(out_d[half, :, t0:t0 + 2, :],
                                          outr[:])

            emit_gate_h(0)
            emit_gate_g(0)
            emit_gate_h(1)
            emit_gate_g(1)
            apply_a(0)
            for pg in range(N_PAIRS):
                if pg + 2 < N_PAIRS:
                    emit_gate_h(pg + 2)
                if pg + 1 < N_PAIRS and pg + 1 >= 2:
                    emit_gate_g(pg + 1)
                if pg + 1 < N_PAIRS:
                    apply_a(pg + 1)
                apply_b(pg)
            ps2.__exit__(None, None, None)

    nc.compile()
    return nc


_NC_CACHE = None


def _get_module():
    global _NC_CACHE
    if _NC_CACHE is None:
        _NC_CACHE = _build_module()
    return _NC_CACHE


def _fold_weights(inputs):
    f32 = np.float32
    gam = inputs["rms_gamma"].astype(f32)
    s_lin = 1.0 / np.sqrt(C)
    a0 = (gam[:, None] * inputs["W_lin0"] * s_lin).astype(BF16)
    a1 = ((gam[:, None] * inputs["W_lin1"] * s_lin) @ inputs["w_cross"].T
          * (1.0 / np.sqrt(2.0 * C))).astype(BF16)
    a2 = ((gam[:, None] * inputs["W_lin2"] * s_lin) @ inputs["w_dot"].T
          * (1.0 / np.sqrt(3.0 * C))).astype(BF16)
    g1 = (inputs["gate_W1"] / inputs["std_inv"][:, None]).astype(BF16)
    b1r = ((-inputs["mean_inv"] / inputs["std_inv"]) @ inputs["gate_W1"]).astype(f32)
    b1 = np.ascontiguousarray(b1r.reshape(2, INV).T).astype(f32)   # [INV, 2]
    g2 = np.ascontiguousarray(inputs["gate_W2"].reshape(2, INV, K)).astype(BF16)
    wo = (inputs["ln_gamma"][:, None] * inputs["W_out"]).astype(BF16)
    return a0, a1, a2, g1, b1, g2, wo


def _onehot_consts():
    # ohc[:, t, m] = 1.0 iff m == t : lhsT [C, 32] slab t sums partitions
    # into stats row t (other rows accumulate +0)
    ohc = np.zeros((C, N_TILES * N_TILES), np.float32)
    for t in range(N_TILES):
        ohc[:, t * N_TILES + t] = 1.0
    # ohr[g, t*C + c] = v if g == t : lhsT broadcasting row t of rhs
    ohr_mu = np.zeros((N_TILES, N_TILES * C), np.float32)
    ohr_r = np.zeros((N_TILES, N_TILES * C), np.float32)
    for t in range(N_TILES):
        ohr_mu[t, t * C:(t + 1) * C] = -1.0 / K
        ohr_r[t, t * C:(t + 1) * C] = 1.0
    return ohc.astype(BF16), ohr_mu.astype(BF16), ohr_r.astype(BF16)


_PERM = np.concatenate([
    np.arange(INV),
    INV + 3 * np.arange(C),
    INV + 1 + 3 * np.arange(C),
    INV + 2 + 3 * np.arange(C),
])


def _make_in_maps(inputs):
    a0, a1, a2, g1, b1, g2, wo = _fold_weights(inputs)
    ohc, ohr_mu, ohr_r = _onehot_consts()

    x = np.asarray(inputs["atomic_embeddings"], dtype=np.float32)
    x_cm = x.T[_PERM]                                  # [512, N_ATOMS] view

    consts = {"a0": a0, "a1": a1, "a2": a2, "g1": g1, "b1": b1,
              "g2": g2, "wo": wo, "ohc": ohc,
              "ohr_mu": ohr_mu, "ohr_r": ohr_r}
    in_maps = []
    for c in range(N_CORES):
        m = dict(consts)
        shard = x_cm[:, c * N_SHARD:(c + 1) * N_SHARD]         # [4*C, N_SHARD]
        # -> [C, N_TILES, 4, TILE] bf16: per partition, per tile, 4KB contig
        m["x_cm"] = np.ascontiguousarray(
            shard.reshape(4, C, N_TILES, TILE).transpose(1, 2, 0, 3)).astype(BF16)
        in_maps.append(m)
    return in_maps


def kernel(**inputs):
    nc = _get_module()
    in_maps = _make_in_maps(inputs)
    res = run_bass_kernel_spmd(nc, in_maps, core_ids=list(range(N_CORES)))
    outs = []
    for r in res.results:
        o = r["out"]                                   # [2, C, N_TILES, TILE]
        outs.append(o.transpose(2, 3, 0, 1).reshape(N_SHARD, OUT))
    return np.ascontiguousarray(np.concatenate(outs, axis=0)).astype(np.float32)
